# revision 21
# baseline (speedup 1.0000x reference)
"""Trainium2 Bass kernel for the KNet-style recurrent chain (batch=1), v3.

Distribution: FC2 tensor-parallel across 8 cores (5760 W2a rows + the
matching W2b columns per core; host sums the 8 partial y's + b2b); the
small GRU/FC chain is replicated on every core.

Memory strategy (the kernel is DMA-bound):
  - GRU/small-FC weights in fp8 e4m3, scaled x32 on host (their ~0.02
    magnitudes would land in e4m3's subnormal range unscaled); the 1/32
    descale is folded into each drain (ACT scale= / DVE scalar-mul).
    Activations stay bf16 (mixed bf16xfp8 matmul).  FC2 stays bf16
    (fp8 FC2 fails the 2e-2 accuracy gate).
  - All weights PRE-SWIZZLED on host into the exact SBUF tile layout
    [128, nk, nsz] so every weight DMA is one contiguous flat copy.
  - All biases are FOLDED INTO THE WEIGHTS via designated always-1.0
    pad slots (slot = element M of each 128-padded vector).  Inputs get
    the 1.0 from the host; FC outputs via relu(1*1); GRU outputs via a
    sigmoid(14)=1 entry in the z-gate pad column (h'[576] = z*h = 1).
  - Matvec psums [1,<=512] drain on ACT (fused sigmoid/relu + descale)
    or DVE; PE transposes (bf16, 4-byte-aligned psum cols) rebuild
    P-layout; GRU pointwise runs on [128,5] P-layout tiles.
  - FC2a is split into an hSig-half and an hS-half: the hSig-half MMs
    run DURING the GRU_S phase (its 12 partial rows park in SBUF), so
    the final FC2 phase only runs the hS-half + add + FC2b.  Both
    halves' weights are SBUF-resident; FC2b streams through a ring on
    the scalar DMA queue, interleaved with the hS-pass.
"""

import sys

sys.path.insert(0, "/opt/trn_rl_repo")

import numpy as np
import ml_dtypes

BF16 = ml_dtypes.bfloat16
FP8 = ml_dtypes.float8_e4m3fn
F32 = np.float32
W8SCALE = 32.0

NCORES = 8
H = 576
D2_HID, D2_IN, D2_OUT = 46080, 1152, 576
MSH = D2_HID // NCORES
NM2 = MSH // 128
FCB_GRP = 3

# matvec specs: name -> (seg names, Kp, Mp, act)
MV = {
    "fc5":  (["x5"], 128, 512, "relu"),
    "q_rz": (["out5", "h_q"], 1152, 1280, "sigmoid"),
    "q_in": (["out5"], 512, 640, None),
    "q_hn": (["h_q"], 640, 640, None),
    "fc6":  (["x6"], 128, 512, "relu"),
    "sig_rz": (["hQ", "out6", "h_sig"], 1792, 1280, "sigmoid"),
    "sig_in": (["hQ", "out6"], 1152, 640, None),
    "sig_hn": (["h_sig"], 640, 640, None),
    "fc1":  (["hSig"], 640, 640, "relu"),
    "fc7":  (["obs"], 128, 1024, "relu"),
    "s_rz": (["out1", "out7", "h_s"], 2304, 1280, "sigmoid"),
    "s_in": (["out1", "out7"], 1664, 640, None),
    "s_hn": (["h_s"], 640, 640, None),
}
SEG_COLS = {
    "x5": 1, "x6": 1, "obs": 1,
    "out5": 4, "out6": 4, "out7": 8, "out1": 5,
    "h_q": 5, "h_sig": 5, "h_s": 5,
    "hQ": 5, "hSig": 5, "hS": 5,
}
# columns of the merged const tensor [128, 18]
CONST_COLS = {"x5": (0, 1), "x6": (1, 1), "obs": (2, 1),
              "h_q": (3, 5), "h_sig": (8, 5), "h_s": (13, 5)}

_CACHE = {}


def _stripes(mp):
    return [(n0, min(512, mp - n0)) for n0 in range(0, mp, 512)]


def _build_program():
    import concourse.bass as bass  # noqa: F401
    from concourse import bacc, mybir
    import concourse.tile as tile

    f32 = mybir.dt.float32
    bf16 = mybir.dt.bfloat16
    fp8 = mybir.dt.float8e4
    AF = mybir.ActivationFunctionType

    nc = bacc.Bacc(
        "TRN2", target_bir_lowering=False, debug=False, num_devices=NCORES
    )

    def din(name, shape, dt):
        return nc.dram_tensor(name, list(shape), dt, kind="ExternalInput")

    d_const = din("consts", (128, 18), bf16)
    d_w = {}
    for name, (seg_names, kp, mp, act) in MV.items():
        nk = kp // 128
        for si, (n0, nsz) in enumerate(_stripes(mp)):
            d_w[f"{name}_s{si}"] = din(f"{name}_s{si}", (128, nk, nsz), fp8)
    for si, (n0, nsz) in enumerate(_stripes(MSH)):
        d_w[f"fc2a_sig_s{si}"] = din(f"fc2a_sig_s{si}", (128, 5, nsz), bf16)
        d_w[f"fc2a_hs_s{si}"] = din(f"fc2a_hs_s{si}", (128, 5, nsz), bf16)
    for g in range(NM2 // FCB_GRP):
        d_w[f"fc2b_g{g}"] = din(f"fc2b_g{g}", (128, FCB_GRP, D2_OUT), bf16)
    d_y = nc.dram_tensor("y", [1, D2_OUT], f32, kind="ExternalOutput")

    with tile.TileContext(nc) as tc:
        with (
            tc.tile_pool(name="const", bufs=1) as constp,
            tc.tile_pool(name="vecs", bufs=1) as vecp,
            tc.tile_pool(name="rows", bufs=1) as rowp,
            tc.tile_pool(name="gw", bufs=6) as gwp,
            tc.tile_pool(name="fc2a", bufs=1) as fc2ap,
            tc.tile_pool(name="w2bp", bufs=4) as w2bp,
            tc.tile_pool(name="ps", bufs=1, space="PSUM") as psp,
        ):
            ct = constp.tile([128, 18], bf16, name="t_consts", tag="t_consts")
            nc.sync.dma_start(out=ct, in_=d_const[:])
            # segs: name -> (tile, base col)
            segs = {k: (ct, c0) for k, (c0, _) in CONST_COLS.items()}
            ident = constp.tile([1, 1], bf16, name="ident", tag="ident")
            nc.vector.memset(ident, 1.0)
            # preload ACT LUTs (sigmoid/tanh) while the first weights DMA
            warm = constp.tile([1, 1], f32, name="warm", tag="warm")
            nc.scalar.activation(warm, ident, AF.Sigmoid)
            nc.scalar.activation(warm, ident, AF.Tanh)

            # FC2a resident tiles; triggers go on the SCALAR dma queue,
            # woven through the chain, so a stalled GRU-ring trigger on
            # the sync queue never blocks FC2a prefetch (head-of-line).
            fc2a_sig_tiles = []
            fc2a_hs_tiles = []
            _sig_pending = list(enumerate(_stripes(MSH)))
            _hs_pending = list(enumerate(_stripes(MSH)))

            def drop_sig(n):
                for _ in range(n):
                    if not _sig_pending:
                        return
                    si, (n0, nsz) = _sig_pending.pop(0)
                    t = fc2ap.tile([128, 5, nsz], bf16,
                                   name=f"w_fc2as_{si}", tag=f"fc2as_{si}")
                    nc.scalar.dma_start(out=t, in_=d_w[f"fc2a_sig_s{si}"][:])
                    fc2a_sig_tiles.append(t)

            def drop_hs(n):
                for _ in range(n):
                    if not _hs_pending:
                        return
                    si, (n0, nsz) = _hs_pending.pop(0)
                    t = fc2ap.tile([128, 5, nsz], bf16,
                                   name=f"w_fc2ah_{si}", tag=f"fc2ah_{si}")
                    nc.scalar.dma_start(out=t, in_=d_w[f"fc2a_hs_s{si}"][:])
                    fc2a_hs_tiles.append(t)

            def seg_cols(names):
                cols = []
                for s in names:
                    t, base = segs[s]
                    for j in range(SEG_COLS[s]):
                        cols.append((t, base + j))
                return cols

            def emit_matvec(name):
                seg_names, kp, mp, act = MV[name]
                nk = kp // 128
                cols = seg_cols(seg_names)
                assert len(cols) == nk, (name, len(cols), nk)
                row = rowp.tile([1, mp], bf16, name=f"row_{name}",
                                tag=f"row_{name}")
                for si, (n0, nsz) in enumerate(_stripes(mp)):
                    wt = gwp.tile([128, nk, nsz], fp8, tag="gw",
                                  name=f"w_{name}_{si}", bufs=3)
                    nc.sync.dma_start(out=wt, in_=d_w[f"{name}_s{si}"][:])
                    ps = psp.tile([1, 512], f32, tag="mv", bufs=3,
                                  name=f"ps_{name}_{si}")
                    for c, (st, j) in enumerate(cols):
                        nc.tensor.matmul(
                            ps[0:1, 0:nsz], st[:, j : j + 1],
                            wt[:, c, 0:nsz],
                            start=(c == 0), stop=(c == nk - 1),
                            skip_group_check=True,
                        )
                    dst = row[0:1, n0 : n0 + nsz]
                    if act == "sigmoid":
                        nc.scalar.activation(dst, ps[0:1, 0:nsz], AF.Sigmoid,
                                             scale=1.0 / W8SCALE)
                    elif act == "relu":
                        nc.scalar.activation(dst, ps[0:1, 0:nsz], AF.Relu,
                                             scale=1.0 / W8SCALE)
                    else:
                        nc.vector.tensor_scalar_mul(dst, ps[0:1, 0:nsz],
                                                    1.0 / W8SCALE)
                return row

            def emit_transposes(tp, row, ncols, col0, n_done, n_total):
                for c in range(ncols):
                    nc.tensor.matmul(
                        tp[:, col0 + c, 0:1],
                        row[0:1, c * 128 : (c + 1) * 128], ident,
                        is_transpose=True,
                        start=(n_done + c == 0),
                        stop=(n_done + c == n_total - 1),
                        skip_group_check=True,
                    )
                return n_done + ncols

            def do_fc(name, out_name):
                row = emit_matvec(name)
                nc_ = MV[name][2] // 128
                tp = psp.tile([128, 20, 2], bf16, tag="tp", bufs=2,
                              name=f"tp_{name}")
                emit_transposes(tp, row, nc_, 0, 0, nc_)
                out = vecp.tile([128, nc_], bf16, name=out_name,
                                tag=out_name)
                nc.vector.tensor_copy(out, tp[:, 0:nc_, 0])
                segs[out_name] = (out, 0)

            def do_gru(g, h_name, out_name):
                row_rz = emit_matvec(f"{g}_rz")
                row_in = emit_matvec(f"{g}_in")
                row_hn = emit_matvec(f"{g}_hn")
                tp = psp.tile([128, 20, 2], bf16, tag="tp", bufs=2,
                              name=f"tp_{g}")
                n = emit_transposes(tp, row_rz, 10, 0, 0, 20)
                n = emit_transposes(tp, row_in, 5, 10, n, 20)
                emit_transposes(tp, row_hn, 5, 15, n, 20)
                rzc = vecp.tile([128, 10], bf16, name=f"rzc_{g}", tag="rzc",
                                bufs=2)
                gh = vecp.tile([128, 10], bf16, name=f"gh_{g}", tag="gh",
                               bufs=2)
                nc.vector.tensor_copy(rzc, tp[:, 0:10, 0])
                nc.vector.tensor_copy(gh, tp[:, 10:20, 0])
                t1 = vecp.tile([128, 5], f32, name=f"t1_{g}", tag="t1",
                               bufs=2)
                nt = vecp.tile([128, 5], f32, name=f"nt_{g}", tag="nt",
                               bufs=2)
                t3 = vecp.tile([128, 5], f32, name=f"t3_{g}", tag="t3",
                               bufs=2)
                h_tile, hb = segs[h_name]
                nc.vector.tensor_mul(t1, rzc[:, 0:5], gh[:, 5:10])
                nc.vector.tensor_add(t1, gh[:, 0:5], t1)
                nc.scalar.activation(nt, t1, AF.Tanh)
                nc.vector.tensor_sub(t3, h_tile[:, hb : hb + 5], nt)
                nc.vector.tensor_mul(t3, rzc[:, 5:10], t3)
                out = vecp.tile([128, 5], bf16, name=out_name, tag=out_name)
                nc.vector.tensor_add(out, nt, t3)
                segs[out_name] = (out, 0)

            # ---- the chain ----
            drop_sig(2)
            do_fc("fc5", "out5")
            drop_sig(2)
            drop_hs(1)
            do_gru("q", "h_q", "hQ")
            drop_sig(3)
            drop_hs(1)
            do_fc("fc6", "out6")
            drop_sig(2)
            drop_hs(1)
            do_gru("sig", "h_sig", "hSig")
            drop_sig(3)
            drop_hs(2)
            do_fc("fc1", "out1")
            drop_hs(2)
            do_fc("fc7", "out7")
            drop_hs(5)
            assert not _sig_pending

            # ---- FC2a pass 1: hSig half, overlapped with GRU_S ----
            sig_cols = seg_cols(["hSig"])
            partials = []
            for si, (n0, nsz) in enumerate(_stripes(MSH)):
                drop_hs(1)
                wt = fc2a_sig_tiles[si]
                ps = psp.tile([1, 512], f32, tag="mv", bufs=3,
                              name=f"ps_f1_{si}")
                for c, (st, j) in enumerate(sig_cols):
                    nc.tensor.matmul(
                        ps[0:1, 0:nsz], st[:, j : j + 1], wt[:, c, 0:nsz],
                        start=(c == 0), stop=(c == 4),
                        skip_group_check=True,
                    )
                part = rowp.tile([1, 512], f32, name=f"part_{si}",
                                 tag=f"part_{si}")
                nc.vector.tensor_copy(part[0:1, 0:nsz], ps[0:1, 0:nsz])
                partials.append(part)

            do_gru("s", "h_s", "hS")
            assert not _hs_pending

            # FC2b ring: all triggers on the (now idle) sync queue; a
            # trigger for group g+5 stalls sync until group g's MMs
            # finish, but nothing else is queued behind it except the
            # final y store.
            fcb_tiles = []
            for gidx in range(NM2 // FCB_GRP):
                wtb = w2bp.tile([128, FCB_GRP, D2_OUT], bf16, tag="w2b",
                                name=f"w2b_{gidx}", bufs=4)
                nc.sync.dma_start(out=wtb, in_=d_w[f"fc2b_g{gidx}"][:])
                fcb_tiles.append(wtb)

            # ---- FC2a pass 2 (hS half) + FC2b, pipelined ----
            hs_cols = seg_cols(["hS"])
            ps_hfc = psp.tile([128, NM2, 2], bf16, tag="hfc", bufs=1,
                              name="ps_hfc")
            h_fc = vecp.tile([128, NM2], bf16, name="h_fc", tag="h_fc")
            ps_y = psp.tile([1, D2_OUT], f32, tag="y", bufs=1, name="ps_y")
            ntp = 0
            fcb_done = 0
            for si, (n0, nsz) in enumerate(_stripes(MSH)):
                wt = fc2a_hs_tiles[si]
                ps = psp.tile([1, 512], f32, tag="mv", bufs=3,
                              name=f"ps_f2_{si}")
                for c, (st, j) in enumerate(hs_cols):
                    nc.tensor.matmul(
                        ps[0:1, 0:nsz], st[:, j : j + 1], wt[:, c, 0:nsz],
                        start=(c == 0), stop=(c == 4),
                        skip_group_check=True,
                    )
                tmp = rowp.tile([1, 512], f32, name=f"tmp_{si}", tag="tmp",
                                bufs=2)
                nc.vector.tensor_add(tmp[0:1, 0:nsz], ps[0:1, 0:nsz],
                                     partials[si][0:1, 0:nsz])
                hstr = rowp.tile([1, 512], bf16, name=f"hstr_{si}",
                                 tag="hstr", bufs=2)
                nc.scalar.activation(hstr[0:1, 0:nsz], tmp[0:1, 0:nsz],
                                     AF.Relu)
                nb = nsz // 128
                for c in range(nb):
                    nc.tensor.matmul(
                        ps_hfc[:, 4 * si + c, 0:1],
                        hstr[0:1, c * 128 : (c + 1) * 128], ident,
                        is_transpose=True,
                        start=(ntp == 0), stop=(ntp == NM2 - 1),
                        skip_group_check=True,
                    )
                    ntp += 1
                nc.vector.tensor_copy(
                    h_fc[:, 4 * si : 4 * si + nb],
                    ps_hfc[:, 4 * si : 4 * si + nb, 0],
                )
                navail = 4 * si + nb
                while (fcb_done + 1) * FCB_GRP <= navail:
                    gidx = fcb_done
                    wtb = fcb_tiles[gidx]
                    for j in range(FCB_GRP):
                        kb = gidx * FCB_GRP + j
                        lhs = h_fc[:, kb : kb + 1]
                        nc.tensor.matmul(
                            ps_y[0:1, 0:512], lhs, wtb[:, j, 0:512],
                            start=(kb == 0), stop=(kb == NM2 - 1),
                            skip_group_check=True,
                        )
                        nc.tensor.matmul(
                            ps_y[0:1, 512:576], lhs, wtb[:, j, 512:576],
                            start=(kb == 0), stop=(kb == NM2 - 1),
                            skip_group_check=True,
                        )
                    fcb_done += 1
            assert fcb_done == NM2 // FCB_GRP

            y_sb = constp.tile([1, D2_OUT], f32, name="y_sb", tag="y_sb")
            nc.vector.tensor_copy(y_sb, ps_y[0:1, 0:D2_OUT])
            nc.sync.dma_start(out=d_y[:], in_=y_sb)

    nc.compile()
    return nc


def _get_program():
    if "nc" not in _CACHE:
        _CACHE["nc"] = _build_program()
    return _CACHE["nc"]


# ----------------------------------------------------------------------------
# host-side data prep
# ----------------------------------------------------------------------------


def _play_cols(v, ncols, one_slot=None):
    v = np.asarray(v, F32).ravel()
    buf = np.zeros((ncols, 128), F32)
    buf.reshape(-1)[: v.size] = v
    if one_slot is not None:
        buf.reshape(-1)[one_slot] = 1.0
    return buf.T  # [128, ncols] fp32


def _rz_cols(w):
    out = np.zeros((w.shape[0], 1280), F32)
    out[:, 0:576] = w[:, 0:576]
    out[:, 640:1216] = w[:, 576:1152]
    return out


def _swizzle(m, wp, kp, mp, dt=FP8, scale=W8SCALE):
    nk = kp // 128
    w3 = (wp * scale).reshape(nk, 128, mp).transpose(1, 0, 2)
    out = {}
    for si, (n0, nsz) in enumerate(_stripes(mp)):
        out[f"{m}_s{si}"] = np.ascontiguousarray(
            w3[:, :, n0 : n0 + nsz].astype(dt)
        )
    return out


def _gru_host(tag, Wih, Whh, bih, bhh, xsegs):
    out = {}
    kp_rz = sum(p for _, p in xsegs) + 640
    wp = np.zeros((kp_rz, 1280), F32)
    r0 = p0 = 0
    for rows, pad in xsegs:
        wp[p0 : p0 + rows] = _rz_cols(Wih[0:1152, r0 : r0 + rows].T)
        r0 += rows
        p0 += pad
    wp[p0 : p0 + 576] = _rz_cols(Whh[0:1152].T)
    brow = _rz_cols((bih[0:1152] + bhh[0:1152])[None, :])
    brow[0, 1216] = 7.5      # sigmoid(7.5) -> 1.0 in bf16; 7.5*32=240
    # stays below e4m3 exp-15 encodings (>=256), which some decoders
    # treat as inf/nan
    wp[p0 + 576] = brow
    out.update(_swizzle(f"{tag}_rz", wp, kp_rz, 1280))

    kp_in = sum(p for _, p in xsegs)
    wp = np.zeros((kp_in, 640), F32)
    r0 = p0 = 0
    for rows, pad in xsegs:
        wp[p0 : p0 + rows, 0:576] = Wih[1152:1728, r0 : r0 + rows].T
        r0 += rows
        p0 += pad
    wp[xsegs[0][0], 0:576] = bih[1152:1728]
    out.update(_swizzle(f"{tag}_in", wp, kp_in, 640))

    wp = np.zeros((640, 640), F32)
    wp[0:576, 0:576] = Whh[1152:1728].T
    wp[576, 0:576] = bhh[1152:1728]
    out.update(_swizzle(f"{tag}_hn", wp, 640, 640))
    return out


def _fc_host(tag, W, b, kreal, kp, mp, one_col=None):
    wp = np.zeros((kp, mp), F32)
    m = W.shape[0]
    wp[0:kreal, 0:m] = W.T
    wp[kreal, 0:m] = b
    if one_col is not None:
        wp[kreal, one_col] = 1.0
    return _swizzle(tag, wp, kp, mp)


def _prep_inputs(inputs):
    g = {k: np.asarray(v, F32) for k, v in inputs.items()}

    consts = np.zeros((128, 18), F32)
    consts[:, 0:1] = _play_cols(g["fw_evol_diff"], 1, one_slot=24)
    consts[:, 1:2] = _play_cols(g["fw_update_diff"], 1, one_slot=24)
    consts[:, 2:3] = _play_cols(
        np.concatenate([g["obs_diff"], g["obs_innov_diff"]]), 1, one_slot=48
    )
    consts[:, 3:8] = _play_cols(g["h_Q"], 5, one_slot=576)
    consts[:, 8:13] = _play_cols(g["h_Sigma"], 5, one_slot=576)
    consts[:, 13:18] = _play_cols(g["h_S"], 5, one_slot=576)
    common = {"consts": np.ascontiguousarray(consts).astype(BF16)}

    common.update(_fc_host("fc5", g["W5"], g["b5"], 24, 128, 512,
                           one_col=480))
    common.update(_fc_host("fc6", g["W6"], g["b6"], 24, 128, 512,
                           one_col=480))
    common.update(_fc_host("fc7", g["W7"], g["b7"], 48, 128, 1024,
                           one_col=960))
    common.update(_fc_host("fc1", g["W1"], g["b1"], 576, 640, 640,
                           one_col=576))
    common.update(_gru_host("q", g["Wih_Q"], g["Whh_Q"], g["bih_Q"],
                            g["bhh_Q"], [(480, 512)]))
    common.update(_gru_host("sig", g["Wih_Sig"], g["Whh_Sig"],
                            g["bih_Sig"], g["bhh_Sig"],
                            [(576, 640), (480, 512)]))
    common.update(_gru_host("s", g["Wih_S"], g["Whh_S"], g["bih_S"],
                            g["bhh_S"], [(576, 640), (960, 1024)]))

    w2aT = g["W2a"].T
    w2bT = g["W2b"].T
    in_maps = []
    for k in range(NCORES):
        m = dict(common)
        sl = slice(k * MSH, (k + 1) * MSH)
        wp = np.zeros((640, MSH), F32)
        wp[0:576] = w2aT[0:576, sl]
        wp[576] = g["b2a"][sl]
        m.update(_swizzle("fc2a_sig", wp, 640, MSH, dt=BF16, scale=1.0))
        wp = np.zeros((640, MSH), F32)
        wp[0:576] = w2aT[576:1152, sl]
        m.update(_swizzle("fc2a_hs", wp, 640, MSH, dt=BF16, scale=1.0))
        wb3 = w2bT[sl].reshape(NM2, 128, D2_OUT).transpose(1, 0, 2)
        for gi in range(NM2 // FCB_GRP):
            m[f"fc2b_g{gi}"] = np.ascontiguousarray(
                wb3[:, gi * FCB_GRP : (gi + 1) * FCB_GRP, :].astype(BF16)
            )
        in_maps.append(m)
    return in_maps


def run(trace=False, **inputs):
    from concourse.bass_utils import run_bass_kernel_spmd

    nc = _get_program()
    in_maps = _prep_inputs(inputs)
    res = run_bass_kernel_spmd(nc, in_maps, list(range(NCORES)), trace=trace)
    y = np.zeros(D2_OUT, np.float64)
    for r in res.results:
        y += r["y"].reshape(-1).astype(np.float64)
    out = (y.astype(F32) + np.asarray(inputs["b2b"], F32)).reshape(24, 24)
    return out, res


def kernel(**inputs):
    out, _ = run(trace=False, **inputs)
    return out
